# revision 2
# baseline (speedup 1.0000x reference)
"""Bidirectional-LSTM Trainium2 kernel (nn_BLSTM).

Problem: B=64,T=512,D=H=512. Two independent LSTMs (forward input x_f,
backward input x_b, both scanned t=0..T-1), outputs summed, then two
H x H linear layers (no nonlinearity between them -> collapsed into one
matmul with W21 = W2 @ W1, b21 = W2 @ b1 + b2).

Sharding (8 cores, fully SPMD - same program, different data):
  core r: direction = r % 2 (0 -> f, 1 -> b), batch shard = r // 2
  each core runs one LSTM direction for 16 batches, then applies the
  fused linear; the host sums the per-direction partial outputs.

On-core layout ("gate-major"): hidden/gate dim on partitions, batch on
the free dim, so the recurrent matmul is lhsT=WhhT tiles [128k x 128j],
rhs = hT [128k x 16b], psum gates [128, 256] with columns
[sig-group: i | f | o || tanh-group: g], each gate block (hc,b) packed.
Input projections xg = x @ WihT (+bias) are computed chunk-ahead and
interleaved into the PE stream as filler between recurrence steps.
"""

import functools
import numpy as np
import ml_dtypes

import concourse.bass as bass
import concourse.tile as tile
from concourse import bacc, mybir
from concourse.bass_utils import run_bass_kernel_spmd

# ---------------- problem constants ----------------
B, T, D, H = 64, 512, 512, 512
G = 4 * H                 # 2048 gate dim
N_CORES = 8
BL = B // (N_CORES // 2)  # 16 local batch per core
TC = 32                   # timesteps per chunk
NCH = T // TC             # chunks
# psum gate-block order: sigmoid block first (i,f,o) then tanh block (g)
BLK = {0: 0, 1: 1, 2: 3, 3: 2}   # torch gate idx (i,f,g,o) -> psum block

WEIGHT_DTYPE = "float32"   # 'float32' | 'bfloat16'  (weights + streamed acts)

F32 = mybir.dt.float32
AFT = mybir.ActivationFunctionType


def _dt():
    wdt = F32 if WEIGHT_DTYPE == "float32" else mybir.dt.bfloat16
    return wdt


def _np_wdt():
    return np.float32 if WEIGHT_DTYPE == "float32" else ml_dtypes.bfloat16


def _build_program():
    wdt = _dt()
    nc = bacc.Bacc("TRN2", target_bir_lowering=False, debug=False,
                   num_devices=N_CORES)

    xT_d = nc.dram_tensor("xT", [4, 128, T, BL], wdt, kind="ExternalInput").ap()
    wih_d = nc.dram_tensor("wih", [4, 128, G], wdt, kind="ExternalInput").ap()
    whh_d = nc.dram_tensor("whh", [4, 128, G], wdt, kind="ExternalInput").ap()
    w21_d = nc.dram_tensor("w21", [4, 128, H], wdt, kind="ExternalInput").ap()
    biasg_d = nc.dram_tensor("biasg", [128, 16], F32, kind="ExternalInput").ap()
    b21_d = nc.dram_tensor("b21", [128, 4], F32, kind="ExternalInput").ap()
    h0_d = nc.dram_tensor("h0p", [128, 64], wdt, kind="ExternalInput").ap()
    c0_d = nc.dram_tensor("c0p", [128, 64], F32, kind="ExternalInput").ap()
    pred_d = nc.dram_tensor("predT", [H, T * BL], F32, kind="ExternalOutput").ap()

    with tile.TileContext(nc) as tc:
        with (
            tc.tile_pool(name="const", bufs=1) as cpool,
            tc.tile_pool(name="xch", bufs=2) as xch_pool,
            tc.tile_pool(name="xg", bufs=2) as xg_pool,
            tc.tile_pool(name="ring", bufs=2) as ring_pool,
            tc.tile_pool(name="gates", bufs=3) as gates_pool,
            tc.tile_pool(name="acts", bufs=3) as acts_pool,
            tc.tile_pool(name="small", bufs=4) as small_pool,
            tc.tile_pool(name="cstate", bufs=2) as c_pool,
            tc.tile_pool(name="evac", bufs=2) as evac_pool,
            tc.tile_pool(name="gps", bufs=2, space="PSUM") as gps_pool,
            tc.tile_pool(name="pps", bufs=2, space="PSUM") as pps_pool,
            tc.tile_pool(name="lps", bufs=2, space="PSUM") as lps_pool,
        ):
            # ---- preload constants ----
            whh_sb = cpool.tile([128, 4 * G], wdt, tag="whh")
            wih_sb = cpool.tile([128, 4 * G], wdt, tag="wih")
            w21_sb = cpool.tile([128, 4 * H], wdt, tag="w21")
            biasg_sb = cpool.tile([128, 16], F32, tag="biasg")
            b21_sb = cpool.tile([128, 4], F32, tag="b21")
            h0_sb = cpool.tile([128, 64], wdt, tag="h0")
            c0_sb = cpool.tile([128, 64], F32, tag="c0")
            for kc in range(4):
                nc.gpsimd.dma_start(whh_sb[:, kc * G:(kc + 1) * G], whh_d[kc])
                nc.gpsimd.dma_start(wih_sb[:, kc * G:(kc + 1) * G], wih_d[kc])
                nc.gpsimd.dma_start(w21_sb[:, kc * H:(kc + 1) * H], w21_d[kc])
            nc.gpsimd.dma_start(biasg_sb[:], biasg_d[:])
            nc.gpsimd.dma_start(b21_sb[:], b21_d[:])
            nc.gpsimd.dma_start(h0_sb[:], h0_d[:])
            nc.gpsimd.dma_start(c0_sb[:], c0_d[:])

            # ---- projection helpers ----
            def proj_dma(ch):
                xch = xch_pool.tile([128, 4 * TC * BL], wdt, tag="xch")
                for dc in range(4):
                    nc.gpsimd.dma_start(
                        xch[:, dc * TC * BL:(dc + 1) * TC * BL],
                        xT_d[dc, :, ch * TC:(ch + 1) * TC, :])
                return xch

            def proj_group(xch, xg, jc):
                """xg[:, t*256 + off(jc) : +16] = x @ WihT block + bias."""
                g_idx, hc = jc // 4, jc % 4
                off = BLK[g_idx] * 64 + hc * 16
                pp = pps_pool.tile([128, TC * BL], F32, tag="pp")
                for dc in range(4):
                    nc.tensor.matmul(
                        pp[:],
                        wih_sb[:, dc * G + jc * 128: dc * G + (jc + 1) * 128],
                        xch[:, dc * TC * BL:(dc + 1) * TC * BL],
                        start=(dc == 0), stop=(dc == 3))
                dst = xg[:].rearrange("p (t c) -> p t c", c=256)[:, :, off:off + 16]
                nc.scalar.activation(dst, pp[:], AFT.Identity,
                                     bias=biasg_sb[:, jc:jc + 1])

            def linear_group(ring_src, ch_src, jc):
                lp = lps_pool.tile([128, TC * BL], F32, tag="lp")
                r3 = ring_src[:].rearrange("p (t c) -> p t c", c=64)
                for kc in range(4):
                    nc.tensor.matmul(
                        lp[:],
                        w21_sb[:, kc * H + jc * 128: kc * H + (jc + 1) * 128],
                        r3[:, :, kc * 16:(kc + 1) * 16],
                        start=(kc == 0), stop=(kc == 3))
                ev = evac_pool.tile([128, TC * BL], F32, tag="ev")
                nc.scalar.activation(ev[:], lp[:], AFT.Identity,
                                     bias=b21_sb[:, jc:jc + 1])
                nc.gpsimd.dma_start(
                    pred_d[jc * 128:(jc + 1) * 128,
                           ch_src * TC * BL:(ch_src + 1) * TC * BL], ev[:])

            # ---- prologue: project chunk 0 ----
            xch = proj_dma(0)
            xg_cur = xg_pool.tile([128, TC * 256], F32, tag="xg")
            for jc in range(16):
                proj_group(xch, xg_cur, jc)

            c_prev = c0_sb
            prev_ring = None
            xg_next = None
            for ch in range(NCH):
                ring = ring_pool.tile([128, TC * 64], wdt, tag="ring")
                for tl in range(TC):
                    if tl > 0:
                        hsrc, hoff = ring, (tl - 1) * 64
                    elif ch > 0:
                        hsrc, hoff = prev_ring, (TC - 1) * 64
                    else:
                        hsrc, hoff = h0_sb, 0
                    # ---- recurrence matmuls ----
                    gps = gps_pool.tile([128, 256], F32, tag="gps")
                    for jc in range(16):
                        g_idx, hc = jc // 4, jc % 4
                        off = BLK[g_idx] * 64 + hc * 16
                        for kc in range(4):
                            nc.tensor.matmul(
                                gps[:, off:off + 16],
                                whh_sb[:, kc * G + jc * 128: kc * G + (jc + 1) * 128],
                                hsrc[:, hoff + kc * 16: hoff + (kc + 1) * 16],
                                start=(kc == 0), stop=(kc == 3))
                    # ---- interleaved filler work on PE ----
                    if ch + 1 < NCH:
                        if tl == 0:
                            xch = proj_dma(ch + 1)
                            xg_next = xg_pool.tile([128, TC * 256], F32, tag="xg")
                        if tl % 2 == 0:
                            proj_group(xch, xg_next, tl // 2)
                    if ch >= 1 and tl in (3, 11, 19, 27):
                        linear_group(prev_ring, ch - 1, (tl - 3) // 8)
                    # ---- gate nonlinearities + state update ----
                    gates = gates_pool.tile([128, 256], F32, tag="gates")
                    nc.vector.tensor_add(gates[:], gps[:],
                                         xg_cur[:, tl * 256:(tl + 1) * 256])
                    acts = acts_pool.tile([128, 256], F32, tag="acts")
                    nc.scalar.activation(acts[:, 0:192], gates[:, 0:192],
                                         AFT.Sigmoid)
                    nc.scalar.activation(acts[:, 192:256], gates[:, 192:256],
                                         AFT.Tanh)
                    t1 = small_pool.tile([128, 64], F32, tag="t1")
                    nc.vector.tensor_mul(t1[:], acts[:, 0:64], acts[:, 192:256])
                    c_new = c_pool.tile([128, 64], F32, tag="c")
                    nc.vector.tensor_mul(c_new[:], acts[:, 64:128], c_prev[:])
                    nc.vector.tensor_add(c_new[:], c_new[:], t1[:])
                    tnc = small_pool.tile([128, 64], F32, tag="tnc")
                    nc.scalar.activation(tnc[:], c_new[:], AFT.Tanh)
                    nc.vector.tensor_mul(ring[:, tl * 64:(tl + 1) * 64],
                                         acts[:, 128:192], tnc[:])
                    c_prev = c_new
                prev_ring = ring
                if ch + 1 < NCH:
                    xg_cur = xg_next
            # epilogue: linear for the last chunk
            for jc in range(4):
                linear_group(prev_ring, NCH - 1, jc)

    nc.compile()
    return nc


@functools.lru_cache(maxsize=1)
def _get_program():
    return _build_program()


def _pack_core_inputs(x, h0, c0, Wih, Whh, bias, W21, b21_or_zero):
    """Host-side layout prep for one core. x:[BL,T,D], h0/c0:[BL,H]."""
    npw = _np_wdt()
    xT = np.ascontiguousarray(
        x.transpose(2, 1, 0).reshape(4, 128, T, BL)).astype(npw)
    wih = np.ascontiguousarray(Wih.T.reshape(4, 128, G)).astype(npw)
    whh = np.ascontiguousarray(Whh.T.reshape(4, 128, G)).astype(npw)
    w21 = np.ascontiguousarray(W21.T.reshape(4, 128, H)).astype(npw)
    biasg = np.ascontiguousarray(bias.reshape(16, 128).T).astype(np.float32)
    b21v = np.ascontiguousarray(b21_or_zero.reshape(4, 128).T).astype(np.float32)
    h0p = np.ascontiguousarray(
        h0.T.reshape(4, 128, BL).transpose(1, 0, 2).reshape(128, 64)).astype(npw)
    c0p = np.ascontiguousarray(
        c0.T.reshape(4, 128, BL).transpose(1, 0, 2).reshape(128, 64)).astype(np.float32)
    return {"xT": xT, "wih": wih, "whh": whh, "w21": w21, "biasg": biasg,
            "b21": b21v, "h0p": h0p, "c0p": c0p}


def kernel(x_f, x_b, h0_f, c0_f, h0_b, c0_b,
           Wih_f, Whh_f, bih_f, bhh_f,
           Wih_b, Whh_b, bih_b, bhh_b,
           W1, b1, W2, b2):
    x_f, x_b = np.asarray(x_f, np.float32), np.asarray(x_b, np.float32)
    h0_f, c0_f = np.asarray(h0_f, np.float32), np.asarray(c0_f, np.float32)
    h0_b, c0_b = np.asarray(h0_b, np.float32), np.asarray(c0_b, np.float32)
    Wih_f, Whh_f = np.asarray(Wih_f, np.float32), np.asarray(Whh_f, np.float32)
    Wih_b, Whh_b = np.asarray(Wih_b, np.float32), np.asarray(Whh_b, np.float32)
    bias_f = np.asarray(bih_f, np.float32) + np.asarray(bhh_f, np.float32)
    bias_b = np.asarray(bih_b, np.float32) + np.asarray(bhh_b, np.float32)
    W1, b1 = np.asarray(W1, np.float32), np.asarray(b1, np.float32)
    W2, b2 = np.asarray(W2, np.float32), np.asarray(b2, np.float32)

    W21 = (W2 @ W1).astype(np.float32)          # pred = out @ W21.T + b21
    b21 = (W2 @ b1 + b2).astype(np.float32)
    zeros = np.zeros_like(b21)

    in_maps = []
    for r in range(N_CORES):
        d, s = r % 2, r // 2
        sl = slice(s * BL, (s + 1) * BL)
        if d == 0:
            in_maps.append(_pack_core_inputs(
                x_f[sl], h0_f[sl], c0_f[sl], Wih_f, Whh_f, bias_f, W21, b21))
        else:
            in_maps.append(_pack_core_inputs(
                x_b[sl], h0_b[sl], c0_b[sl], Wih_b, Whh_b, bias_b, W21, zeros))

    nc = _get_program()
    res = run_bass_kernel_spmd(nc, in_maps, core_ids=list(range(N_CORES)))

    out = np.empty((B, T, H), np.float32)
    for s in range(N_CORES // 2):
        sT = res.results[2 * s]["predT"] + res.results[2 * s + 1]["predT"]
        out[s * BL:(s + 1) * BL] = sT.reshape(H, T, BL).transpose(2, 1, 0)
    return out.reshape(B * T, H)


# revision 9
# speedup vs baseline: 1917.6061x; 1917.6061x over previous
"""Bidirectional-LSTM Trainium2 kernel (nn_BLSTM).

Problem: B=64,T=512,D=H=512. Two independent LSTMs (forward input x_f,
backward input x_b, both scanned t=0..T-1), outputs summed, then two
H x H linear layers (no nonlinearity between them -> collapsed into one
matmul with W21 = W2 @ W1, b21 = W2 @ b1 + b2).

Sharding (8 cores, fully SPMD - same program, different data):
  core r: direction = r % 2 (0 -> f, 1 -> b), batch shard = r // 2
  each core runs one LSTM direction for 16 batches, then applies the
  fused linear; the host sums the per-direction partial outputs.

On-core layout ("gate-major"): hidden/gate dim on partitions, batch on
the free dim, so the recurrent matmul is lhsT=WhhT tiles [128k x 128j],
rhs = hT [128k x 16b], psum gates [128, 256] with columns
[sig-group: i | f | o || tanh-group: g], each gate block (hc,b) packed.
Input projections xg = x @ WihT (+bias) are computed chunk-ahead and
interleaved into the PE stream as filler between recurrence steps.
"""

import functools
import os
import numpy as np
import ml_dtypes

import concourse.bass as bass
import concourse.tile as tile
from concourse import bacc, mybir
from concourse.bass_utils import run_bass_kernel_spmd

# ---------------- problem constants ----------------
B, T, D, H = 64, 512, 512, 512
G = 4 * H                 # 2048 gate dim
N_CORES = 8
BL = B // (N_CORES // 2)  # 16 local batch per core
TC = 32                   # timesteps per chunk
NCH = T // TC             # chunks
NCH_BUILD = int(os.environ.get("K_CHUNKS", NCH))
TANH_AS_SIG = os.environ.get("K_TANH_SIG", "0") == "1"
COLTILE = int(os.environ.get("K_COLTILE", "1"))   # 1 or 4: col-tiled recurrence MMs
# psum gate-block order: sigmoid block first (i,f,o) then tanh block (g)
BLK = {0: 0, 1: 1, 2: 3, 3: 2}   # torch gate idx (i,f,g,o) -> psum block

WEIGHT_DTYPE = "bfloat16"   # 'float32' | 'bfloat16'  (weights + streamed acts)

F32 = mybir.dt.float32
AFT = mybir.ActivationFunctionType


def _dt():
    wdt = F32 if WEIGHT_DTYPE == "float32" else mybir.dt.bfloat16
    return wdt


def _np_wdt():
    return np.float32 if WEIGHT_DTYPE == "float32" else ml_dtypes.bfloat16


def _build_program(chunks=None, tanh_sig=None):
    if chunks is None:
        chunks = NCH_BUILD
    if tanh_sig is None:
        tanh_sig = TANH_AS_SIG
    wdt = _dt()
    nc = bacc.Bacc("TRN2", target_bir_lowering=False, debug=False,
                   num_devices=N_CORES)

    xT_d = nc.dram_tensor("xT", [4, 128, T, BL], wdt, kind="ExternalInput").ap()
    wih_d = nc.dram_tensor("wih", [4, 128, G], wdt, kind="ExternalInput").ap()
    whh_d = nc.dram_tensor("whh", [4, 128, G], wdt, kind="ExternalInput").ap()
    w21_d = nc.dram_tensor("w21", [4, 128, H], wdt, kind="ExternalInput").ap()
    biasg_d = nc.dram_tensor("biasg", [128, 16], F32, kind="ExternalInput").ap()
    b21_d = nc.dram_tensor("b21", [128, 4], F32, kind="ExternalInput").ap()
    h0_d = nc.dram_tensor("h0p", [128, 64], wdt, kind="ExternalInput").ap()
    c0_d = nc.dram_tensor("c0p", [128, 64], F32, kind="ExternalInput").ap()
    pred_d = nc.dram_tensor("predT", [H, T * BL], F32, kind="ExternalOutput").ap()

    with tile.TileContext(nc) as tc:
        with (
            tc.tile_pool(name="const", bufs=1) as cpool,
            tc.tile_pool(name="xch", bufs=2) as xch_pool,
            tc.tile_pool(name="xg", bufs=2) as xg_pool,
            tc.tile_pool(name="ring", bufs=2) as ring_pool,
            tc.tile_pool(name="gates", bufs=3) as gates_pool,
            tc.tile_pool(name="acts", bufs=3) as acts_pool,
            tc.tile_pool(name="small", bufs=4) as small_pool,
            tc.tile_pool(name="cstate", bufs=2) as c_pool,
            tc.tile_pool(name="evac", bufs=2) as evac_pool,
            tc.tile_pool(name="gps", bufs=2, space="PSUM") as gps_pool,
            tc.tile_pool(name="pps", bufs=2, space="PSUM") as pps_pool,
            tc.tile_pool(name="lps", bufs=2, space="PSUM") as lps_pool,
        ):
            # ---- preload constants ----
            whh_sb = cpool.tile([128, 4 * G], wdt, tag="whh")
            wih_sb = cpool.tile([128, 4 * G], wdt, tag="wih")
            w21_sb = cpool.tile([128, 4 * H], wdt, tag="w21")
            biasg_sb = cpool.tile([128, 16], F32, tag="biasg")
            b21_sb = cpool.tile([128, 4], F32, tag="b21")
            h0_sb = cpool.tile([128, 64], wdt, tag="h0")
            c0_sb = cpool.tile([128, 64], F32, tag="c0")
            for kc in range(4):
                nc.gpsimd.dma_start(whh_sb[:, kc * G:(kc + 1) * G], whh_d[kc])
                nc.gpsimd.dma_start(wih_sb[:, kc * G:(kc + 1) * G], wih_d[kc])
                nc.gpsimd.dma_start(w21_sb[:, kc * H:(kc + 1) * H], w21_d[kc])
            nc.gpsimd.dma_start(biasg_sb[:], biasg_d[:])
            nc.gpsimd.dma_start(b21_sb[:], b21_d[:])
            nc.gpsimd.dma_start(h0_sb[:], h0_d[:])
            nc.gpsimd.dma_start(c0_sb[:], c0_d[:])

            # ---- projection helpers ----
            def proj_dma(ch):
                xch = xch_pool.tile([128, 4 * TC * BL], wdt, tag="xch")
                for dc in range(4):
                    nc.gpsimd.dma_start(
                        xch[:, dc * TC * BL:(dc + 1) * TC * BL],
                        xT_d[dc, :, ch * TC:(ch + 1) * TC, :])
                return xch

            def proj_group(xch, xg, jc):
                """xg[:, t*256 + off(jc) : +16] = x @ WihT block + bias."""
                g_idx, hc = jc // 4, jc % 4
                off = BLK[g_idx] * 64 + hc * 16
                pp = pps_pool.tile([128, TC * BL], F32, tag="pp")
                for dc in range(4):
                    nc.tensor.matmul(
                        pp[:],
                        wih_sb[:, dc * G + jc * 128: dc * G + (jc + 1) * 128],
                        xch[:, dc * TC * BL:(dc + 1) * TC * BL],
                        start=(dc == 0), stop=(dc == 3))
                dst = xg[:].rearrange("p (t c) -> p t c", c=256)[:, :, off:off + 16]
                nc.scalar.activation(dst, pp[:], AFT.Identity,
                                     bias=biasg_sb[:, jc:jc + 1])

            def linear_group(ring_src, ch_src, jc):
                lp = lps_pool.tile([128, TC * BL], F32, tag="lp")
                r3 = ring_src[:].rearrange("p (t c) -> p t c", c=64)
                for kc in range(4):
                    nc.tensor.matmul(
                        lp[:],
                        w21_sb[:, kc * H + jc * 128: kc * H + (jc + 1) * 128],
                        r3[:, :, kc * 16:(kc + 1) * 16],
                        start=(kc == 0), stop=(kc == 3))
                ev = evac_pool.tile([128, TC * BL], F32, tag="ev")
                nc.scalar.activation(ev[:], lp[:], AFT.Identity,
                                     bias=b21_sb[:, jc:jc + 1])
                nc.gpsimd.dma_start(
                    pred_d[jc * 128:(jc + 1) * 128,
                           ch_src * TC * BL:(ch_src + 1) * TC * BL], ev[:])

            # ---- prologue: project chunk 0 ----
            xch = proj_dma(0)
            xg_cur = xg_pool.tile([128, TC * 256], F32, tag="xg")
            for jc in range(16):
                proj_group(xch, xg_cur, jc)

            c_prev = c0_sb
            prev_ring = None
            xg_next = None
            for ch in range(chunks):
                ring = ring_pool.tile([128, TC * 64], wdt, tag="ring")
                for tl in range(TC):
                    if tl > 0:
                        hsrc, hoff = ring, (tl - 1) * 64
                    elif ch > 0:
                        hsrc, hoff = prev_ring, (TC - 1) * 64
                    else:
                        hsrc, hoff = h0_sb, 0
                    # ---- recurrence matmuls ----
                    gps = gps_pool.tile([128, 256], F32, tag="gps")
                    if COLTILE == 1:
                        for jc in range(16):
                            g_idx, hc = jc // 4, jc % 4
                            off = BLK[g_idx] * 64 + hc * 16
                            for kc in range(4):
                                nc.tensor.matmul(
                                    gps[:, off:off + 16],
                                    whh_sb[:, kc * G + jc * 128: kc * G + (jc + 1) * 128],
                                    hsrc[:, hoff + kc * 16: hoff + (kc + 1) * 16],
                                    start=(kc == 0), stop=(kc == 3))
                    else:
                        # single start=True per step: the first MM clears the
                        # bank's has_written bits; all later MMs overwrite where
                        # unset / accumulate where set, so interleaved per-column
                        # accumulation groups stay correct.
                        w = 128 // COLTILE
                        first = True
                        for jc in range(16):
                            g_idx, hc = jc // 4, jc % 4
                            off = BLK[g_idx] * 64 + hc * 16
                            for kc in range(4):
                                for cg in range(COLTILE):
                                    nc.tensor.matmul(
                                        gps[cg * w:(cg + 1) * w, off:off + 16],
                                        whh_sb[:, kc * G + jc * 128 + cg * w:
                                               kc * G + jc * 128 + (cg + 1) * w],
                                        hsrc[:, hoff + kc * 16: hoff + (kc + 1) * 16],
                                        start=first,
                                        stop=(jc == 15 and kc == 3 and cg == COLTILE - 1),
                                        tile_position=(0, cg * w),
                                        skip_group_check=True)
                                    first = False
                    # ---- interleaved filler work on PE ----
                    if ch + 1 < chunks:
                        if tl == 0:
                            xch = proj_dma(ch + 1)
                            xg_next = xg_pool.tile([128, TC * 256], F32, tag="xg")
                        if tl % 2 == 0:
                            proj_group(xch, xg_next, tl // 2)
                    if ch >= 1 and tl in (3, 11, 19, 27):
                        linear_group(prev_ring, ch - 1, (tl - 3) // 8)
                    # ---- gate nonlinearities + state update ----
                    gates = gates_pool.tile([128, 256], F32, tag="gates")
                    nc.vector.tensor_add(gates[:], gps[:],
                                         xg_cur[:, tl * 256:(tl + 1) * 256])
                    # one sigmoid over all 4 gates; g-rows pre-scaled by 2 so
                    # acts_g = sig(2g) and tanh(g) = 2*sig(2g) - 1
                    acts = acts_pool.tile([128, 256], F32, tag="acts")
                    nc.scalar.activation(acts[:], gates[:], AFT.Sigmoid)
                    t1 = small_pool.tile([128, 64], F32, tag="t1")
                    nc.vector.scalar_tensor_tensor(
                        t1[:], acts[:, 192:256], 2.0, acts[:, 0:64],
                        mybir.AluOpType.mult, mybir.AluOpType.mult)
                    nc.vector.tensor_sub(t1[:], t1[:], acts[:, 0:64])
                    c_new = c_pool.tile([128, 64], F32, tag="c")
                    nc.vector.tensor_mul(c_new[:], acts[:, 64:128], c_prev[:])
                    nc.vector.tensor_add(c_new[:], c_new[:], t1[:])
                    # tanh(c) = 2*sig(2c) - 1
                    s2c = small_pool.tile([128, 64], F32, tag="tnc")
                    nc.scalar.activation(s2c[:], c_new[:], AFT.Sigmoid, scale=2.0)
                    hm = small_pool.tile([128, 64], F32, tag="hm")
                    nc.vector.scalar_tensor_tensor(
                        hm[:], s2c[:], 2.0, acts[:, 128:192],
                        mybir.AluOpType.mult, mybir.AluOpType.mult)
                    nc.vector.tensor_sub(ring[:, tl * 64:(tl + 1) * 64],
                                         hm[:], acts[:, 128:192])
                    c_prev = c_new
                prev_ring = ring
                if ch + 1 < chunks:
                    xg_cur = xg_next
            # epilogue: linear for the last chunk
            for jc in range(4):
                linear_group(prev_ring, chunks - 1, jc)

    nc.compile()
    return nc


@functools.lru_cache(maxsize=4)
def _get_program(chunks=None, tanh_sig=None):
    return _build_program(chunks, tanh_sig)


def _pack_core_inputs(x, h0, c0, Wih, Whh, bias, W21, b21_or_zero):
    """Host-side layout prep for one core. x:[BL,T,D], h0/c0:[BL,H]."""
    npw = _np_wdt()
    # g-gate rows [2H,3H) pre-scaled by 2: tanh(g) computed as 2*sig(2g)-1
    Wih = np.concatenate([Wih[:2 * H], Wih[2 * H:3 * H] * 2.0, Wih[3 * H:]])
    Whh = np.concatenate([Whh[:2 * H], Whh[2 * H:3 * H] * 2.0, Whh[3 * H:]])
    bias = np.concatenate([bias[:2 * H], bias[2 * H:3 * H] * 2.0, bias[3 * H:]])
    xT = np.ascontiguousarray(
        x.transpose(2, 1, 0).reshape(4, 128, T, BL)).astype(npw)
    wih = np.ascontiguousarray(Wih.T.reshape(4, 128, G)).astype(npw)
    whh = np.ascontiguousarray(Whh.T.reshape(4, 128, G)).astype(npw)
    w21 = np.ascontiguousarray(W21.T.reshape(4, 128, H)).astype(npw)
    biasg = np.ascontiguousarray(bias.reshape(16, 128).T).astype(np.float32)
    b21v = np.ascontiguousarray(b21_or_zero.reshape(4, 128).T).astype(np.float32)
    h0p = np.ascontiguousarray(
        h0.T.reshape(4, 128, BL).transpose(1, 0, 2).reshape(128, 64)).astype(npw)
    c0p = np.ascontiguousarray(
        c0.T.reshape(4, 128, BL).transpose(1, 0, 2).reshape(128, 64)).astype(np.float32)
    return {"xT": xT, "wih": wih, "whh": whh, "w21": w21, "biasg": biasg,
            "b21": b21v, "h0p": h0p, "c0p": c0p}


def _make_in_maps(inputs):
    f32 = np.float32
    x_f = np.asarray(inputs["x_f"], f32)
    x_b = np.asarray(inputs["x_b"], f32)
    h0_f, c0_f = np.asarray(inputs["h0_f"], f32), np.asarray(inputs["c0_f"], f32)
    h0_b, c0_b = np.asarray(inputs["h0_b"], f32), np.asarray(inputs["c0_b"], f32)
    Wih_f, Whh_f = np.asarray(inputs["Wih_f"], f32), np.asarray(inputs["Whh_f"], f32)
    Wih_b, Whh_b = np.asarray(inputs["Wih_b"], f32), np.asarray(inputs["Whh_b"], f32)
    bias_f = np.asarray(inputs["bih_f"], f32) + np.asarray(inputs["bhh_f"], f32)
    bias_b = np.asarray(inputs["bih_b"], f32) + np.asarray(inputs["bhh_b"], f32)
    W1, b1 = np.asarray(inputs["W1"], f32), np.asarray(inputs["b1"], f32)
    W2, b2 = np.asarray(inputs["W2"], f32), np.asarray(inputs["b2"], f32)

    W21 = (W2 @ W1).astype(f32)          # pred = out @ W21.T + b21
    b21 = (W2 @ b1 + b2).astype(f32)
    zeros = np.zeros_like(b21)

    in_maps = []
    for r in range(N_CORES):
        d, s = r % 2, r // 2
        sl = slice(s * BL, (s + 1) * BL)
        if d == 0:
            in_maps.append(_pack_core_inputs(
                x_f[sl], h0_f[sl], c0_f[sl], Wih_f, Whh_f, bias_f, W21, b21))
        else:
            in_maps.append(_pack_core_inputs(
                x_b[sl], h0_b[sl], c0_b[sl], Wih_b, Whh_b, bias_b, W21, zeros))
    return in_maps


def _assemble(results):
    out = np.empty((B, T, H), np.float32)
    for s in range(N_CORES // 2):
        sT = results[2 * s]["predT"] + results[2 * s + 1]["predT"]
        out[s * BL:(s + 1) * BL] = sT.reshape(H, T, BL).transpose(2, 1, 0)
    return out.reshape(B * T, H)


def kernel(x_f, x_b, h0_f, c0_f, h0_b, c0_b,
           Wih_f, Whh_f, bih_f, bhh_f,
           Wih_b, Whh_b, bih_b, bhh_b,
           W1, b1, W2, b2):
    in_maps = _make_in_maps(dict(
        x_f=x_f, x_b=x_b, h0_f=h0_f, c0_f=c0_f, h0_b=h0_b, c0_b=c0_b,
        Wih_f=Wih_f, Whh_f=Whh_f, bih_f=bih_f, bhh_f=bhh_f,
        Wih_b=Wih_b, Whh_b=Whh_b, bih_b=bih_b, bhh_b=bhh_b,
        W1=W1, b1=b1, W2=W2, b2=b2))
    nc = _get_program()
    res = run_bass_kernel_spmd(nc, in_maps, core_ids=list(range(N_CORES)))
    return _assemble(res.results)
